# revision 1
# baseline (speedup 1.0000x reference)
"""Trainium2 Bass kernel for nn_DeltaSynapse.

I[b,o] = einsum('beo,dbe,deo,dbe->bo', Weff, Xd, delaymap, Wshort+1)
with Weff[b,e,o] = signs[e,o] * (W[e,o]*(1-frac[e,o]) + Wlong[b,e,o]*frac[e,o])

Identity: I[b,o] = sum_e H2[b,e,o] * Weff[b,e,o],
          H2[b,e,o] = sum_d G[d,b,e] * dm[d,e,o],  G = Xd*(Wshort+1).

Hybrid shard: 2 b-halves x 4 o-quarters (no=512/core). Host computes
Weff; the block-diagonal stationary gb is expanded on device from a
packed gpk (DVE) to keep DMA down. Per e-group g of J=16 e's:
  - gb[(d,j),(bb,j')] = G[d, hb*8+bb, g*16+j]*delta_{j,j'}  (expand)
  - H2 matmul: Hp[(bb,j'), o] = gb.T @ dm[:, g-slice]  (512 cols)
  - Z[(bb,j'), (s,o)] = Hp * Weff-tile   (DVE/GpSimd split)
  - Zred: I_ps[8, (s%4,o)] += eh.T @ Z[:, s, :]  (512-col matmul,
      bank keyed by s%4, accumulated across all blocks)
Final: DVE tensor_reduce folds the 4 bank-chunks -> [8, no] -> out.
"""

import os
import sys
import numpy as np

sys.path.insert(0, "/opt/trn_rl_repo")

import ml_dtypes

BF16 = ml_dtypes.bfloat16

# problem constants
D, B, N = 8, 16, 2048
NCORES = 8
OC = 4            # o-quarters
HBS = 2           # b-halves
NO = N // OC      # per-core o-slice width (512)
J = 16            # e's per group
NG = N // J       # e-groups (128)
HB = B // 2       # b per half (8)
C = 8             # groups per DMA block
NB = NG // C      # DMA blocks (16)


def _consts():
    # eh[p=(bb,j'), bb'] = 1 iff bb' == bb  (bb-major partitions)
    eh = np.zeros((128, HB), dtype=np.float32)
    for bb in range(HB):
        eh[bb * J:(bb + 1) * J, bb] = 1.0
    # dmask[p=(d,j), (s, m=(bb,j'))] = delta_{j, j'}, tiled over s
    p = np.arange(128)
    m = np.arange(128)
    dmask = (p[:, None] % J == m[None, :] % J).astype(np.float32)
    dmask = np.tile(dmask.reshape(128, 1, 128), (1, C, 1)).reshape(128, C * 128)
    return eh, dmask


def host_prep(W, Wlong, Wshort, Xd, delaymap, STDP_frac, signs_pre, use_bf16=True):
    """Host-side prep: Weff fusion, packed G, layout transforms, sharding."""
    dt = BF16 if use_bf16 else np.float32
    W = np.asarray(W, np.float32)
    frac = np.asarray(STDP_frac, np.float32)
    signs = np.where(W > 0, np.sign(np.asarray(signs_pre, np.float32))[:, None],
                     np.float32(0.0))
    A = signs * W * (1.0 - frac)
    SF = signs * frac
    Weff = (A[None] + SF[None] * np.asarray(Wlong, np.float32))  # [B,N,N] f32
    G = (np.asarray(Xd, np.float32) *
         (np.asarray(Wshort, np.float32) + 1.0))  # [D,B,N]

    # dm_r[gc, p=(d,j), (s,o)] = dm[d, (gc*C+s)*J+j, oc*NO+o]
    dmf = np.asarray(delaymap, np.float32)
    dm5 = dmf.reshape(D, NB, C, J, N).transpose(1, 0, 3, 2, 4)  # [NB,D,J,C,N]
    dm_oc = []
    for oc in range(OC):
        sl = slice(oc * NO, (oc + 1) * NO)
        dm_oc.append(np.ascontiguousarray(
            dm5[:, :, :, :, sl].reshape(NB, 128, C * NO)).astype(dt))

    # wf[gc, p=(bb,j'), (s,o)] = Weff[hb*HB+bb, (gc*C+s)*J+j', oc*NO+o]
    wf6 = Weff.reshape(HBS, HB, NB, C, J, N).transpose(0, 2, 1, 4, 3, 5)
    # [hb, NB, HB, J, C, N]

    # gpk[gc, p=(d,j), (s,bb)] = G[d, hb*HB+bb, (gc*C+s)*J+j]
    Gr = G.reshape(D, HBS, HB, NB, C, J)  # [d,hb,bb,gc,s,j]
    gpk_h = Gr.transpose(1, 3, 0, 5, 4, 2)  # [hb, gc, d, j, s, bb]

    ins = []
    for core in range(NCORES):
        hb, oc = core // OC, core % OC
        sl = slice(oc * NO, (oc + 1) * NO)
        ins.append({
            "dm": dm_oc[oc],
            "wf": np.ascontiguousarray(
                wf6[hb, :, :, :, :, sl].reshape(NB, 128, C * NO)).astype(dt),
            "gpk": np.ascontiguousarray(
                gpk_h[hb].reshape(NB, 128, C * HB)).astype(dt),
        })
    return ins


def build_nc(use_bf16=True, n_cores=NCORES, no=NO, ng=NG):
    """Build the SPMD Bass program (same on all cores)."""
    import concourse.bass as bass
    import concourse.bacc as bacc
    import concourse.mybir as mybir
    import concourse.tile as tile
    from contextlib import ExitStack

    dt_big = mybir.dt.bfloat16 if use_bf16 else mybir.dt.float32
    f32 = mybir.dt.float32
    nb = ng // C

    nc = bacc.Bacc("TRN2", target_bir_lowering=False, debug=False,
                   num_devices=n_cores)

    dm = nc.declare_dram_parameter("dm", [nb, 128, C * no], dt_big, isOutput=False).ap()
    wf = nc.declare_dram_parameter("wf", [nb, 128, C * no], dt_big, isOutput=False).ap()
    gpk = nc.declare_dram_parameter("gpk", [nb, 128, C * HB], dt_big, isOutput=False).ap()
    out = nc.declare_dram_parameter("out", [HB, no], f32, isOutput=True).ap()

    eh_np, dmask_np = _consts()
    np_dt = BF16 if use_bf16 else np.float32
    eh_dram = nc.inline_tensor(eh_np.astype(np_dt), name="ehc")
    dmask_dram = nc.inline_tensor(dmask_np.astype(np_dt), name="dmaskc")

    def mmdt(ap):
        return ap if use_bf16 else ap.bitcast(mybir.dt.float32r)

    with tile.TileContext(nc) as tc, ExitStack() as ctx:
        res = ctx.enter_context(tc.tile_pool(name="res", bufs=1))
        eh_sb = res.tile([128, HB], dt_big)
        nc.sync.dma_start(out=eh_sb[:, :], in_=eh_dram.ap())
        dmask_sb = res.tile([128, C * 128], dt_big)
        nc.sync.dma_start(out=dmask_sb[:, :], in_=dmask_dram.ap())

        hs_pool = ctx.enter_context(tc.tile_pool(name="hsp", bufs=3))
        dm_pool = ctx.enter_context(tc.tile_pool(name="dmp", bufs=3))
        wf_pool = ctx.enter_context(tc.tile_pool(name="wfp", bufs=3))
        gp_pool = ctx.enter_context(tc.tile_pool(name="gpp", bufs=3))
        gb_pool = ctx.enter_context(tc.tile_pool(name="gbp", bufs=3))
        z_pool = ctx.enter_context(tc.tile_pool(name="zp", bufs=3))
        psum_h = ctx.enter_context(tc.tile_pool(name="psh", bufs=4, space="PSUM"))
        psum_i = ctx.enter_context(tc.tile_pool(name="psi", bufs=1, space="PSUM"))
        out_pool = ctx.enter_context(tc.tile_pool(name="outp", bufs=1))

        # persistent accumulator [8, (s%4,o)] = 4 PSUM banks
        I_ps = psum_i.tile([HB, 4 * no], f32)

        # z-mul engine split: GpSimd ~1.9x slower than DVE; DVE also does
        # the gb expansion (1 op/block).
        GS_SLOTS = (1, 3, 5, 7)

        for gc in range(nb):
            dm_t = dm_pool.tile([128, C * no], dt_big, tag="dm")
            nc.sync.dma_start(out=dm_t[:, :], in_=dm[gc])
            wf_t = wf_pool.tile([128, C * no], dt_big, tag="wf")
            nc.sync.dma_start(out=wf_t[:, :], in_=wf[gc])
            gp_t = gp_pool.tile([128, C * HB], dt_big, tag="gp")
            nc.sync.dma_start(out=gp_t[:, :], in_=gpk[gc])

            # expand gb[p=(d,j), (s, m=(bb,j'))] = gpk[p,(s,bb)]*dmask[p,j']
            gb_t = gb_pool.tile([128, C * 128], dt_big, tag="gb")
            nc.vector.tensor_mul(
                gb_t.rearrange("p (s b j) -> p s b j", s=C, b=HB),
                gp_t.rearrange("p (s b) -> p s b", s=C).unsqueeze(3)
                    .broadcast_to((128, C, HB, J)),
                dmask_sb.rearrange("p (s b j) -> p s b j", s=C, b=HB))

            gb_v = gb_t.rearrange("p (s m) -> p s m", s=C)
            wf_v = wf_t.rearrange("p (s o) -> p s o", s=C)
            Z_t = z_pool.tile([128, C * no], dt_big, tag="z")
            Z_v = Z_t.rearrange("p (s o) -> p s o", s=C)

            for s in range(C):
                Hp = psum_h.tile([128, no], f32, tag="hp")
                so = slice(s * no, (s + 1) * no)
                nc.tensor.matmul(Hp[:, :],
                                 mmdt(gb_v[:, s, :]),
                                 mmdt(dm_t[:, so]),
                                 start=True, stop=True)
                if s in GS_SLOTS:
                    # GpSimd cannot read PSUM: ACT evacuates to SBUF bf16
                    Hs = hs_pool.tile([128, no], dt_big, tag="hs")
                    nc.scalar.copy(Hs[:, :], Hp[:, :])
                    nc.gpsimd.tensor_mul(Z_v[:, s, :], wf_v[:, s, :], Hs[:, :])
                else:
                    nc.vector.tensor_mul(Z_v[:, s, :], wf_v[:, s, :], Hp[:, :])

                nc.tensor.matmul(I_ps[:, (s % 4) * no:(s % 4 + 1) * no],
                                 mmdt(eh_sb[:, :]),
                                 mmdt(Z_v[:, s, :]),
                                 start=(gc == 0 and s < 4),
                                 stop=(gc == nb - 1 and s >= 4))

        # fold bank-chunks: [8, (k,o)] viewed as [8, o, k] -> reduce X
        I_sb = out_pool.tile([HB, no], f32)
        nc.vector.tensor_reduce(I_sb[:, :],
                                I_ps.rearrange("b (k o) -> b o k", k=4),
                                axis=mybir.AxisListType.X,
                                op=mybir.AluOpType.add)
        nc.sync.dma_start(out=out, in_=I_sb[:, :])

    nc.compile()
    return nc


_CACHE = {}


def kernel(W, Wlong, Wshort, Xd, delaymap, STDP_frac, signs_pre):
    from concourse.bass_utils import run_bass_kernel_spmd

    use_bf16 = os.environ.get("DS_FP32", "0") != "1"
    ins = host_prep(W, Wlong, Wshort, Xd, delaymap, STDP_frac, signs_pre, use_bf16)
    key = ("nc", use_bf16)
    if key not in _CACHE:
        _CACHE[key] = build_nc(use_bf16)
    nc = _CACHE[key]
    r = run_bass_kernel_spmd(nc, ins, list(range(NCORES)))
    out_full = np.zeros((B, N), np.float32)
    for core in range(NCORES):
        hb, oc = core // OC, core % OC
        out_full[hb * HB:(hb + 1) * HB, oc * NO:(oc + 1) * NO] = \
            r.results[core]["out"].astype(np.float32)
    return out_full


if __name__ == "__main__":
    pass



# revision 8
# speedup vs baseline: 1.2221x; 1.2221x over previous
"""Trainium2 Bass kernel for nn_DeltaSynapse.

I[b,o] = einsum('beo,dbe,deo,dbe->bo', Weff, Xd, delaymap, Wshort+1)
with Weff[b,e,o] = signs[e,o] * (W[e,o]*(1-frac[e,o]) + Wlong[b,e,o]*frac[e,o])

Identity: I[b,o] = sum_e H2[b,e,o] * Weff[b,e,o],
          H2[b,e,o] = sum_d G[d,b,e] * dm[d,e,o],  G = Xd*(Wshort+1).

Shard: 8 o-slices (no=256/core); each core handles all B=16 batches as
two halves of 8.  This loads delaymap exactly once per core (HBM floor:
wf 16.8MB + dm 8.4MB + gpk 0.5MB ~= 25.7MB/core).

Per e-group g of J=16 e's, per b-half hb (steps of 8 groups = 1 block):
  - gb[(d,j),(j',s,b)] = G[d,hb*8+b, e]*delta_{j,j'}   (DVE/GpSimd expand,
      j'-outer layout so only the outer AP dim broadcasts)
  - H matmul: Hp[(j',b), o] = gb.T @ dm[:, s-slice]   (256 cols, pairs of
      s share one PSUM bank)
  - Z[(j',b),(s,o)] = Hp * Weff-tile   (DVE reads PSUM直接; one pair via
      ACT evac + GpSimd)
  - Zred: I_ps[8, (s%2,o)] += eh.T @ Z[:, 512-slice]  (accumulated over
      all blocks; PE program order software-pipelined one half-step back)
Final: DVE tensor_reduce folds the 2 chunks -> [8, no] per half -> out.
"""

import os
import sys
import numpy as np

sys.path.insert(0, "/opt/trn_rl_repo")

import ml_dtypes

BF16 = ml_dtypes.bfloat16

# problem constants
D, B, N = 8, 16, 2048
NCORES = 8
OC = 8            # o-slices (one per core)
NO = N // OC      # per-core o-slice width (256)
J = 16            # e's per group
NG = N // J       # e-groups (128)
HB = B // 2       # b per half (8)
C = 8             # groups per DMA block
NB = NG // C      # DMA blocks (16)


def _consts():
    # eh[p=(j',b), b'] = 1 iff b == b'   (j'-major partitions)
    eh = np.zeros((128, HB), dtype=np.float32)
    p = np.arange(128)
    eh[p, p % HB] = 1.0
    # dmask_rep[p=(d,j), (j', s, b)] = delta_{j, j'}  (constant, contiguous)
    jp = np.arange(J)
    m = (p[:, None] % J == jp[None, :]).astype(np.float32)  # [128, 16]
    dmask = np.tile(m.reshape(128, J, 1), (1, 1, C * HB)).reshape(128, J * C * HB)
    return eh, dmask


def host_prep(W, Wlong, Wshort, Xd, delaymap, STDP_frac, signs_pre, use_bf16=True):
    """Host-side prep: Weff fusion, packed G, layout transforms, sharding."""
    dt = BF16 if use_bf16 else np.float32
    W = np.asarray(W, np.float32)
    frac = np.asarray(STDP_frac, np.float32)
    signs = np.where(W > 0, np.sign(np.asarray(signs_pre, np.float32))[:, None],
                     np.float32(0.0))
    A = signs * W * (1.0 - frac)
    SF = signs * frac
    Weff = (A[None] + SF[None] * np.asarray(Wlong, np.float32))  # [B,N,N] f32
    G = (np.asarray(Xd, np.float32) *
         (np.asarray(Wshort, np.float32) + 1.0))  # [D,B,N]

    # dm[gc, p=(d,j), (s,o)] = delaymap[d, (gc*C+s)*J+j, oc*NO+o]
    dmf = np.asarray(delaymap, np.float32)
    dm5 = dmf.reshape(D, NB, C, J, N).transpose(1, 0, 3, 2, 4)  # [NB,D,J,C,N]

    # wf[hb, gc, p=(j',b), (s,o)] = Weff[hb*HB+b, (gc*C+s)*J+j', oc*NO+o]
    wf6 = Weff.reshape(2, HB, NB, C, J, N).transpose(0, 2, 4, 1, 3, 5)
    # [hb, NB, J, HB, C, N]  -> p=(j',b) j'-major

    # gpk[hb, gc, p=(d,j), (s,b)] = G[d, hb*HB+b, (gc*C+s)*J+j]
    Gr = G.reshape(D, 2, HB, NB, C, J)  # [d,hb,b,gc,s,j]
    gpk_h = Gr.transpose(1, 3, 0, 5, 4, 2)  # [hb, gc, d, j, s, b]

    ins = []
    for core in range(NCORES):
        oc = core
        sl = slice(oc * NO, (oc + 1) * NO)
        ins.append({
            "dm": np.ascontiguousarray(
                dm5[:, :, :, :, sl].reshape(NB, 128, C * NO)).astype(dt),
            "wf0": np.ascontiguousarray(
                wf6[0, :, :, :, :, sl].reshape(NB, 128, C * NO)).astype(dt),
            "wf1": np.ascontiguousarray(
                wf6[1, :, :, :, :, sl].reshape(NB, 128, C * NO)).astype(dt),
            "gpk": np.ascontiguousarray(
                gpk_h.reshape(2, NB, 128, C * HB)).astype(dt),
        })
    return ins


def build_nc(use_bf16=True, n_cores=NCORES, no=NO):
    """Build the SPMD Bass program (same on all cores)."""
    import concourse.bass as bass
    import concourse.bacc as bacc
    import concourse.mybir as mybir
    import concourse.tile as tile
    from contextlib import ExitStack

    dt_big = mybir.dt.bfloat16 if use_bf16 else mybir.dt.float32
    f32 = mybir.dt.float32
    nb = NB

    nc = bacc.Bacc("TRN2", target_bir_lowering=False, debug=False,
                   num_devices=n_cores)

    dm = nc.declare_dram_parameter("dm", [nb, 128, C * no], dt_big, isOutput=False).ap()
    wf0 = nc.declare_dram_parameter("wf0", [nb, 128, C * no], dt_big, isOutput=False).ap()
    wf1 = nc.declare_dram_parameter("wf1", [nb, 128, C * no], dt_big, isOutput=False).ap()
    gpk = nc.declare_dram_parameter("gpk", [2, nb, 128, C * HB], dt_big, isOutput=False).ap()
    out = nc.declare_dram_parameter("out", [B, no], f32, isOutput=True).ap()
    wf = (wf0, wf1)

    eh_np, dmask_np = _consts()
    np_dt = BF16 if use_bf16 else np.float32
    eh_dram = nc.inline_tensor(eh_np.astype(np_dt), name="ehc")
    dmask_dram = nc.inline_tensor(dmask_np.astype(np_dt), name="dmaskc")

    with tile.TileContext(nc) as tc, ExitStack() as ctx:
        res = ctx.enter_context(tc.tile_pool(name="res", bufs=1))
        eh_sb = res.tile([128, HB], dt_big)
        nc.sync.dma_start(out=eh_sb[:, :], in_=eh_dram.ap())
        dmask_sb = res.tile([128, J * C * HB], dt_big)
        nc.sync.dma_start(out=dmask_sb[:, :], in_=dmask_dram.ap())

        hs_pool = ctx.enter_context(tc.tile_pool(name="hsp", bufs=3))
        dm_pool = ctx.enter_context(tc.tile_pool(name="dmp", bufs=3))
        wf_pool = ctx.enter_context(tc.tile_pool(name="wfp", bufs=6))
        gp_pool = ctx.enter_context(tc.tile_pool(name="gpp", bufs=6))
        gb_pool = ctx.enter_context(tc.tile_pool(name="gbp", bufs=4))
        z_pool = ctx.enter_context(tc.tile_pool(name="zp", bufs=4))
        psum_h = ctx.enter_context(tc.tile_pool(name="psh", bufs=6, space="PSUM"))
        psum_i = ctx.enter_context(tc.tile_pool(name="psi", bufs=1, space="PSUM"))
        out_pool = ctx.enter_context(tc.tile_pool(name="outp", bufs=2))

        # persistent accumulators: [8, (k=2, o)] = 1 PSUM bank per half
        I_ps = [psum_i.tile([HB, 2 * no], f32, name=f"ips{h}", tag=f"ips{h}")
                for h in range(2)]

        # software pipeline: steps = (gc, hb); Zred for step k emitted
        # after the H matmuls of step k+1.
        steps = [(gc, hb) for gc in range(nb) for hb in range(2)]
        pend = None  # (Z_t, hb, gc) awaiting Zred

        for k, (gc, hb) in enumerate(steps):
            if hb == 0:
                dm_t = dm_pool.tile([128, C * no], dt_big, tag="dm")
                nc.sync.dma_start(out=dm_t[:, :], in_=dm[gc])
            wf_t = wf_pool.tile([128, C * no], dt_big, tag="wf")
            nc.scalar.dma_start(out=wf_t[:, :], in_=wf[hb][gc])
            gp_t = gp_pool.tile([128, C * HB], dt_big, tag="gp")
            nc.sync.dma_start(out=gp_t[:, :], in_=gpk[hb, gc])

            # expand gb[p, (s, j', b)] = gpk[p,(s,b)] * delta_{p%16, j'}
            # (s-major layout so the matmul stationary slice is contiguous)
            gb_t = gb_pool.tile([128, J * C * HB], dt_big, tag="gb")
            eng = nc.vector if (k % 2 == 0) else nc.gpsimd
            eng.tensor_mul(
                gb_t.rearrange("p (s j b) -> p j s b", s=C, j=J),
                gp_t.rearrange("p (s b) -> p s b", s=C)
                    .unsqueeze(1).broadcast_to((128, J, C, HB)),
                dmask_sb.rearrange("p (j s b) -> p j s b", j=J, s=C))

            gb_v = gb_t.rearrange("p (s m) -> p s m", s=C)
            wf_v = wf_t.rearrange("p (s o) -> p s o", s=C)
            Z_t = z_pool.tile([128, C * no], dt_big, tag="z")

            hp_tiles = []
            for t in range(C // 2):
                Hp = psum_h.tile([128, 2 * no], f32, tag="hp")
                for i in range(2):
                    s = 2 * t + i
                    nc.tensor.matmul(Hp[:, i * no:(i + 1) * no],
                                     gb_v[:, s, :],
                                     dm_t[:, s * no:(s + 1) * no],
                                     start=True, stop=True)
                hp_tiles.append(Hp)

            # elementwise Z = wf * Hp  (pair tiles of 512)
            for t in range(C // 2):
                Hp = hp_tiles[t]
                so = slice(2 * t * no, (2 * t + 2) * no)
                if t == 3:
                    # ACT evacuates PSUM -> SBUF bf16, GpSimd multiplies
                    Hs = hs_pool.tile([128, 2 * no], dt_big, tag="hs")
                    nc.scalar.copy(Hs[:, :], Hp[:, :])
                    nc.gpsimd.tensor_mul(Z_t[:, so], wf_t[:, so], Hs[:, :])
                else:
                    nc.vector.tensor_mul(Z_t[:, so], wf_t[:, so], Hp[:, :])

            # Zred for the PREVIOUS step (software pipeline, keeps PE fed)
            if pend is not None:
                pZ, phb, pgc = pend
                for t in range(C // 2):
                    nc.tensor.matmul(
                        I_ps[phb][:, :],
                        eh_sb[:, :],
                        pZ[:, 2 * t * no:(2 * t + 2) * no],
                        start=(pgc == 0 and t == 0),
                        stop=(pgc == nb - 1 and t == C // 2 - 1))
            pend = (Z_t, hb, gc)

        # drain the last step's Zred
        pZ, phb, pgc = pend
        for t in range(C // 2):
            nc.tensor.matmul(I_ps[phb][:, :],
                             eh_sb[:, :],
                             pZ[:, 2 * t * no:(2 * t + 2) * no],
                             start=(pgc == 0 and t == 0),
                             stop=(pgc == nb - 1 and t == C // 2 - 1))

        # fold chunks: [8, (k,o)] viewed as [8, o, k] -> reduce X
        for hb in range(2):
            I_sb = out_pool.tile([HB, no], f32, name=f"isb{hb}", tag="isb")
            nc.vector.tensor_reduce(I_sb[:, :],
                                    I_ps[hb].rearrange("b (k o) -> b o k", k=2),
                                    axis=mybir.AxisListType.X,
                                    op=mybir.AluOpType.add)
            nc.sync.dma_start(out=out[hb * HB:(hb + 1) * HB, :], in_=I_sb[:, :])

    nc.compile()
    return nc


_CACHE = {}


def kernel(W, Wlong, Wshort, Xd, delaymap, STDP_frac, signs_pre):
    from concourse.bass_utils import run_bass_kernel_spmd

    use_bf16 = os.environ.get("DS_FP32", "0") != "1"
    ins = host_prep(W, Wlong, Wshort, Xd, delaymap, STDP_frac, signs_pre, use_bf16)
    key = ("nc", use_bf16)
    if key not in _CACHE:
        _CACHE[key] = build_nc(use_bf16)
    nc = _CACHE[key]
    r = run_bass_kernel_spmd(nc, ins, list(range(NCORES)))
    out_full = np.zeros((B, N), np.float32)
    for core in range(NCORES):
        oc = core
        out_full[:, oc * NO:(oc + 1) * NO] = \
            r.results[core]["out"].astype(np.float32)
    return out_full


if __name__ == "__main__":
    pass
